# revision 3
# baseline (speedup 1.0000x reference)
"""F2NetHead Trainium2 kernel (8 NeuronCores, Bass/Tile).

Reference computation (per batch b):
    qog = x @ W_qog.T + b_qog ; Q,O,G = split(qog)
    cq  = silu(conv1d(Q, conv_w, pad=1) + conv_b)          # mixes channels
    l   = (cq @ w_a.T) / sqrt(d)
    attn= softmax(l, axis=seq)
    glob= sum_seq(Q * attn)                                 # [1, d]
    P   = O * glob
    L   = silu(G) * cumsum(P, axis=seq)
    R   = L @ W_out.T + b_out

Sharding: 8 cores = 4 batches x 2 sequence halves. Each core computes
2048 tokens of one batch. The host supplies the x-rows with a 1-token
halo on each side (zero rows at the sequence edges) so the conv needs no
neighbor exchange. The only cross-core communication is a pairwise
AllReduce of 4 small [d] vectors per batch:
    E  = sum_seq exp(l)            (softmax denominator)
    N  = sum_seq Q * exp(l)        (softmax numerator of glob)
    v  = W_O @ (sum of this half's x rows), masked to the first half
(v rides the allreduce because the matvec is linear; the second half's
cumsum offset is glob * (v + T*b_O), i.e. the first half's P-column-sums,
computed without materializing O.)

On-chip layout is feature-major ([d partitions, tokens free]) so every
sequence-axis op (softmax sums, global sum, cumsum) is a free-dim op.
All matmul operands are bf16 (full PE rate, fp32 PSUM accumulation;
~3e-3 rel err on hw — comfortably inside the 2e-2 gate) which halves
DMA traffic and SBUF residency vs fp32r. That slack is spent on:
  * weight prefetch — every phase's weights are DMA'd during phase A/B1
    so no phase start ever stalls on an HBM load;
  * collective overlap — phase C's O- and G-matmuls (which do not
    depend on the allreduce) are emitted before anything that consumes
    the allreduce result, so the PE array chews through them while the
    collective is in flight; only the cheap scan/scale/W_out tail waits.
silu is computed as x*sigmoid(x) (ACT sigmoid + DVE multiply).
"""

import numpy as np

import concourse.bacc as bacc
import concourse.mybir as mybir
import concourse.tile as tile
from concourse.bass_utils import run_bass_kernel_spmd

F32 = mybir.dt.float32
BF16 = mybir.dt.bfloat16
AF = mybir.ActivationFunctionType
OP = mybir.AluOpType

B, S, D, DM = 4, 4096, 1024, 1024
N_CORES = 8
T = S // 2            # tokens per core
TH = T + 2            # with halo
DT = D // 128         # d tiles (8)
KT = DM // 128        # contraction tiles (8)
ABLK = 410            # phase A token block (5 blocks over TH=2050)
BBLK = 512            # phase B token block (4 blocks over T)
CBLK = 256            # phase C token block (8 blocks over T)
NCB = T // CBLK
SCALE = 1.0 / float(np.sqrt(D))


def _emit(tc, nc, prm, phases=5):
    reps = 1
    if phases >= 100:
        reps, phases = phases // 100, 5
    for _ in range(reps):
        _emit_once(tc, nc, prm, phases)


def _emit_once(tc, nc, prm, phases):
    x, wqt, wct, wat, wot = prm["x"], prm["wqt"], prm["wct"], prm["wat"], prm["wot"]
    bq, bo, bg, cb, bout = prm["bq"], prm["bo"], prm["bg"], prm["cb"], prm["bout"]
    hf0, hf1, r_out = prm["hf0"], prm["hf1"], prm["r"]

    with (
        tc.tile_pool(name="cols", bufs=1) as cols,
        tc.tile_pool(name="wbig", bufs=1) as wbig,
        tc.tile_pool(name="dram", bufs=1, space="DRAM") as dram,
    ):
        # phase C / matvec weights — DMA'd later (interleaved with B1's
        # conv-weight loads) so they never delay phase A's startup
        woo = wbig.tile([128, KT, DT, 128], BF16)   # W_O^T
        wog = wbig.tile([128, KT, DT, 128], BF16)   # W_G^T
        wo2 = wbig.tile([128, KT, DT, 128], BF16)   # W_out^T
        wa = wbig.tile([128, KT, DT, 128], BF16)    # w_a^T

        # per-partition bias / flag columns ([128, DT] with d = a*128 + p)
        bq_sb = cols.tile([128, DT], F32)
        bo_sb = cols.tile([128, DT], F32)
        bg_sb = cols.tile([128, DT], F32)
        cb_sb = cols.tile([128, DT], F32)
        bout_sb = cols.tile([128, DT], F32)
        hf0_sb = cols.tile([128, 1], F32)
        hf1_sb = cols.tile([128, 1], F32)
        for t_, d_ in ((bq_sb, bq), (bo_sb, bo), (bg_sb, bg), (cb_sb, cb),
                       (bout_sb, bout), (hf0_sb, hf0), (hf1_sb, hf1)):
            nc.sync.dma_start(t_[:], d_[:])

        # accumulators that survive across phases
        sx_cols = cols.tile([128, KT, 5], F32)      # per-A-block x sums
        e_cols = cols.tile([128, DT * 4], F32)      # per-(a,B-block) exp sums
        n_cols = cols.tile([128, DT * 4], F32)      # per-(a,B-block) Q*exp sums
        stage = cols.tile([128, 4 * DT], F32)       # allreduce staging
        red = cols.tile([128, 4 * DT], F32)         # allreduce result
        glob = cols.tile([128, DT], F32)
        offset = cols.tile([128, DT], F32)

        # ---------------- phase A: Q^T over TH halo'd tokens ----------------
        with tc.tile_pool(name="qt", bufs=1) as qt_pool:
            qt = qt_pool.tile([128, DT, TH], BF16)
            with (
                tc.tile_pool(name="wq", bufs=1) as wq_pool,
                tc.tile_pool(name="xa", bufs=2) as xa_pool,
                tc.tile_pool(name="psa", bufs=8, space="PSUM") as psa,
            ):
                wq = [wq_pool.tile([128, DT, 128], BF16, tag=f"wq{kc}",
                                   name=f"wq{kc}") for kc in range(KT)]
                for kc in range(KT):
                    nc.sync.dma_start(
                        wq[kc][:],
                        wqt[kc * 128:(kc + 1) * 128, 0:D]
                        .rearrange("p (a m) -> p a m", m=128),
                    )
                for blk in range(5):
                    t0 = blk * ABLK
                    xt = [xa_pool.tile([128, ABLK], BF16, tag=f"xa{kc}",
                                       name=f"xa{kc}") for kc in range(KT)]
                    for kc in range(KT):
                        nc.sync.dma_start(
                            xt[kc][:],
                            x[kc * 128:(kc + 1) * 128, t0:t0 + ABLK],
                        )
                    # x column-sums over main (non-halo) tokens for cumsum offset
                    lo = 1 - t0 if t0 < 1 else 0
                    hi = ABLK - max(0, t0 + ABLK - (TH - 1))
                    for kc in range(KT):
                        nc.vector.tensor_reduce(
                            sx_cols[:, kc, blk:blk + 1], xt[kc][:, lo:hi],
                            axis=mybir.AxisListType.X, op=OP.add,
                        )
                    for a in range(DT):
                        ps = psa.tile([128, ABLK], F32, tag="ps")
                        for kc in range(KT):
                            nc.tensor.matmul(
                                ps[:], wq[kc][:, a, :], xt[kc][:],
                                start=(kc == 0), stop=(kc == KT - 1),
                            )
                        nc.vector.tensor_scalar_add(
                            qt[:, a, t0:t0 + ABLK], ps[:], bq_sb[:, a:a + 1]
                        )

            # ------------- phase B1: cq^T = silu(conv(Q)) -------------
            with tc.tile_pool(name="cq", bufs=1) as cq_pool:
                cq = cq_pool.tile([128, DT, T], BF16)
                with (
                    tc.tile_pool(name="wc", bufs=2) as wc_pool,
                    tc.tile_pool(name="psb", bufs=8, space="PSUM") as psb,
                ):
                    for a in range(DT):
                        wc = wc_pool.tile([128, 3, KT, 128], BF16, tag="wc")
                        for k3 in range(3):
                            nc.sync.dma_start(
                                wc[:, k3, :, :],
                                wct[k3, :, a * 128:(a + 1) * 128]
                                .rearrange("(kc p) m -> p kc m", p=128),
                            )
                        # spread the later phases' weight loads between the
                        # conv-weight loads so neither ever stalls compute
                        if a < 4:
                            w_pre = (wa, woo, wog, wo2)[a]
                            for kc in range(KT):
                                if a == 0:
                                    src = wat[kc * 128:(kc + 1) * 128, :]
                                elif a == 1:
                                    src = wqt[kc * 128:(kc + 1) * 128, D:2 * D]
                                elif a == 2:
                                    src = wqt[kc * 128:(kc + 1) * 128, 2 * D:3 * D]
                                else:
                                    src = wot[kc * 128:(kc + 1) * 128, :]
                                nc.sync.dma_start(
                                    w_pre[:, kc, :, :],
                                    src.rearrange("p (a m) -> p a m", m=128),
                                )
                        for blk in range(T // BBLK):
                            t0 = blk * BBLK
                            ps = psb.tile([128, BBLK], F32, tag="ps")
                            first = True
                            for k3 in range(3):
                                for kc in range(KT):
                                    nc.tensor.matmul(
                                        ps[:], wc[:, k3, kc, :],
                                        qt[:, kc, t0 + k3:t0 + k3 + BBLK],
                                        start=first,
                                        stop=(k3 == 2 and kc == KT - 1),
                                    )
                                    first = False
                            sig = wc_pool.tile([128, BBLK], F32, tag="sig")
                            nc.scalar.activation(
                                sig[:], ps[:], AF.Sigmoid, bias=cb_sb[:, a:a + 1]
                            )
                            nc.vector.scalar_tensor_tensor(
                                cq[:, a, t0:t0 + BBLK], ps[:], cb_sb[:, a:a + 1],
                                sig[:], OP.add, OP.mult,
                            )

                # ------- phase B2: E/N partial sums from exp(logits) -------
                with (
                    tc.tile_pool(name="ex", bufs=2) as ex_pool,
                    tc.tile_pool(name="psl", bufs=8, space="PSUM") as psl,
                ):
                    for blk in range(T // BBLK):
                        t0 = blk * BBLK
                        for a in range(DT):
                            ps = psl.tile([128, BBLK], F32, tag="ps")
                            for kc in range(KT):
                                nc.tensor.matmul(
                                    ps[:], wa[:, kc, a, :], cq[:, kc, t0:t0 + BBLK],
                                    start=(kc == 0), stop=(kc == KT - 1),
                                )
                            expl = ex_pool.tile([128, BBLK], F32, tag="expl")
                            idx = a * 4 + blk
                            nc.scalar.activation(
                                expl[:], ps[:], AF.Exp, scale=SCALE,
                                accum_out=e_cols[:, idx:idx + 1],
                            )
                            prod = ex_pool.tile([128, BBLK], F32, tag="prod")
                            nc.vector.scalar_tensor_tensor(
                                prod[:], expl[:], 0.0,
                                qt[:, a, t0 + 1:t0 + 1 + BBLK],
                                OP.add, OP.mult,
                                accum_out=n_cols[:, idx:idx + 1],
                            )

        # ------- allreduce staging: E, N, v = W_O @ (masked x sums) -------
        nc.vector.tensor_reduce(
            stage[:, 0:DT], e_cols[:].rearrange("p (a b) -> p a b", b=4),
            axis=mybir.AxisListType.X, op=OP.add,
        )
        nc.vector.tensor_reduce(
            stage[:, DT:2 * DT], n_cols[:].rearrange("p (a b) -> p a b", b=4),
            axis=mybir.AxisListType.X, op=OP.add,
        )
        # x sums (main tokens) masked to the first half: sxm = sx * (1-h)
        sxm = cols.tile([128, KT], BF16)
        sxf = cols.tile([128, KT], F32)
        nc.vector.tensor_reduce(
            sxf[:], sx_cols[:], axis=mybir.AxisListType.X, op=OP.add,
        )
        nc.vector.tensor_scalar_mul(sxm[:], sxf[:], hf0_sb[:, 0:1])
        # v = W_O @ sxm — linear, so it rides the allreduce (the matvec
        # would otherwise gate the post-collective critical path)
        with tc.tile_pool(name="psm", bufs=2, space="PSUM") as psm:
            for a in range(DT):
                ps = psm.tile([128, 1], F32, tag="ps")
                for kc in range(KT):
                    nc.tensor.matmul(
                        ps[:], woo[:, kc, a, :], sxm[:, kc:kc + 1],
                        start=(kc == 0), stop=(kc == KT - 1),
                    )
                nc.vector.tensor_scalar_add(
                    stage[:, 3 * DT + a:3 * DT + a + 1], ps[:], 0.0
                )

        if phases == 99:
            # timing-model variant: skip the collective (TimelineSim
            # cannot model collectives); copy stage -> red locally
            nc.vector.tensor_copy(red[:], stage[:])
        else:
            cc_in = dram.tile([128, 4 * DT], F32)
            cc_out = dram.tile([128, 4 * DT], F32)
            nc.sync.dma_start(cc_in[:], stage[:])
            nc.gpsimd.collective_compute(
                "AllReduce", OP.add,
                replica_groups=[[0, 1], [2, 3], [4, 5], [6, 7]],
                ins=[cc_in.opt()], outs=[cc_out.opt()],
            )
            nc.sync.dma_start(red[:], cc_out[:])

        # ---------------- phase C: O,G -> P -> cumsum -> L -> R ----------------
        # Part 1 (emitted before anything consumes `red`): O/G matmuls and
        # silu(G), staged to SBUF in bf16. The PE array runs these while the
        # allreduce is in flight.
        with (
            tc.tile_pool(name="stg", bufs=1) as stg_pool,
            tc.tile_pool(name="xc", bufs=2) as xc_pool,
            tc.tile_pool(name="blkb", bufs=2) as blk_pool,
            tc.tile_pool(name="psc", bufs=8, space="PSUM") as psc,
        ):
            ot = stg_pool.tile([128, DT, T], BF16)
            gt = stg_pool.tile([128, DT, T], BF16)
            for blk in range(NCB):
                t0 = blk * CBLK
                xt = xc_pool.tile([128, KT, CBLK], BF16, tag="xc")
                for kc in range(KT):
                    nc.sync.dma_start(
                        xt[:, kc, :],
                        x[kc * 128:(kc + 1) * 128, t0 + 1:t0 + 1 + CBLK],
                    )
                for a in range(DT):
                    ps = psc.tile([128, CBLK], F32, tag="ps")
                    for kc in range(KT):
                        nc.tensor.matmul(
                            ps[:], woo[:, kc, a, :], xt[:, kc, :],
                            start=(kc == 0), stop=(kc == KT - 1),
                        )
                    nc.scalar.copy(ot[:, a, t0:t0 + CBLK], ps[:])
                for a in range(DT):
                    ps = psc.tile([128, CBLK], F32, tag="ps")
                    for kc in range(KT):
                        nc.tensor.matmul(
                            ps[:], wog[:, kc, a, :], xt[:, kc, :],
                            start=(kc == 0), stop=(kc == KT - 1),
                        )
                    sig = xc_pool.tile([128, CBLK], F32, tag="sig")
                    nc.scalar.activation(
                        sig[:], ps[:], AF.Sigmoid, bias=bg_sb[:, a:a + 1]
                    )
                    nc.vector.scalar_tensor_tensor(
                        gt[:, a, t0:t0 + CBLK], ps[:], bg_sb[:, a:a + 1], sig[:],
                        OP.add, OP.mult,
                    )

            # Part 2 (consumes `red`): glob, cumsum offsets, then per block
            # P -> cumsum -> L -> R. Only scan/scale work plus the W_out
            # matmuls sit behind the collective.
            recip = cols.tile([128, DT], F32)
            nc.vector.reciprocal(recip[:], red[:, 0:DT])
            nc.vector.tensor_mul(glob[:], red[:, DT:2 * DT], recip[:])
            bo_t = cols.tile([128, DT], F32)
            nc.vector.tensor_scalar_mul(bo_t[:], bo_sb[:], float(T))
            offv = cols.tile([128, DT], F32)
            nc.vector.tensor_add(offv[:], red[:, 3 * DT:4 * DT], bo_t[:])
            nc.vector.tensor_mul(offset[:], offv[:], glob[:])
            nc.vector.tensor_scalar_mul(offset[:], offset[:], hf1_sb[:, 0:1])
            boglob = cols.tile([128, DT], F32)
            nc.vector.tensor_mul(boglob[:], bo_sb[:], glob[:])

            c_prev = None
            for blk in range(NCB):
                t0 = blk * CBLK
                pt = blk_pool.tile([128, DT, CBLK], F32, tag="pt")
                ct = blk_pool.tile([128, DT, CBLK], F32, tag="ct")
                carry = xc_pool.tile([128, DT], F32, tag="carry")
                lt = blk_pool.tile([128, DT, CBLK], BF16, tag="lt")
                rt = blk_pool.tile([128, DT, CBLK], BF16, tag="rt")
                for a in range(DT):
                    # P = (O + b_o) * glob = O*glob + (b_o*glob), on ACT
                    nc.scalar.activation(
                        pt[:, a, :], ot[:, a, t0:t0 + CBLK], AF.Identity,
                        bias=boglob[:, a:a + 1], scale=glob[:, a:a + 1],
                    )
                    init = (offset[:, a:a + 1] if c_prev is None
                            else c_prev[:, a:a + 1])
                    nc.vector.tensor_tensor_scan(
                        ct[:, a, :], pt[:, a, :], pt[:, a, :], init,
                        OP.add, OP.bypass,
                    )
                # carry the last cumsum column via ACT so the next
                # block's scan does not read a scan output directly
                nc.scalar.copy(carry[:], ct[:, :, CBLK - 1:CBLK])
                for a in range(DT):
                    nc.vector.tensor_mul(
                        lt[:, a, :], gt[:, a, t0:t0 + CBLK], ct[:, a, :]
                    )
                for a in range(DT):
                    ps = psc.tile([128, CBLK], F32, tag="ps")
                    for kc in range(KT):
                        nc.tensor.matmul(
                            ps[:], wo2[:, kc, a, :], lt[:, kc, :],
                            start=(kc == 0), stop=(kc == KT - 1),
                        )
                    nc.scalar.activation(
                        rt[:, a, :], ps[:], AF.Identity,
                        bias=bout_sb[:, a:a + 1],
                    )
                for a in range(DT):
                    nc.sync.dma_start(
                        r_out[a * 128:(a + 1) * 128, t0:t0 + CBLK],
                        rt[:, a, :],
                    )
                c_prev = carry


_CACHE = {}


def _build(phases=5):
    if phases in _CACHE:
        return _CACHE[phases]
    nc = bacc.Bacc(None, target_bir_lowering=False, num_devices=N_CORES)
    prm = {
        "x": nc.declare_dram_parameter("x", [DM, TH], BF16, isOutput=False),
        "wqt": nc.declare_dram_parameter("wqt", [DM, 3 * D], BF16, isOutput=False),
        "wct": nc.declare_dram_parameter("wct", [3, D, D], BF16, isOutput=False),
        "wat": nc.declare_dram_parameter("wat", [D, D], BF16, isOutput=False),
        "wot": nc.declare_dram_parameter("wot", [D, D], BF16, isOutput=False),
        "bq": nc.declare_dram_parameter("bq", [128, DT], F32, isOutput=False),
        "bo": nc.declare_dram_parameter("bo", [128, DT], F32, isOutput=False),
        "bg": nc.declare_dram_parameter("bg", [128, DT], F32, isOutput=False),
        "cb": nc.declare_dram_parameter("cb", [128, DT], F32, isOutput=False),
        "bout": nc.declare_dram_parameter("bout", [128, DT], F32, isOutput=False),
        "hf0": nc.declare_dram_parameter("hf0", [128, 1], F32, isOutput=False),
        "hf1": nc.declare_dram_parameter("hf1", [128, 1], F32, isOutput=False),
        "r": nc.declare_dram_parameter("r", [DM, T], BF16, isOutput=True),
    }
    with tile.TileContext(nc, num_cores=N_CORES) as tc:
        _emit(tc, nc, prm, phases)
    nc.compile()
    _CACHE[phases] = nc
    return nc


def make_in_maps(x, W_qog, b_qog, conv_w, conv_b, w_a, W_out, b_out):
    f = np.float32
    bf = mybir.dt.np(BF16)
    x = np.asarray(x, f)
    wqt = np.ascontiguousarray(np.asarray(W_qog, f).T.astype(bf))   # [dm, 3d]
    wct = np.ascontiguousarray(np.asarray(conv_w, f).transpose(2, 1, 0).astype(bf))
    wat = np.ascontiguousarray(np.asarray(w_a, f).T.astype(bf))
    wot = np.ascontiguousarray(np.asarray(W_out, f).T.astype(bf))

    def col(v):  # [d] -> [128, DT] with d = a*128 + p
        return np.ascontiguousarray(np.asarray(v, f).reshape(DT, 128).T)

    b_qog = np.asarray(b_qog, f)
    bq, bo, bg = col(b_qog[:D]), col(b_qog[D:2 * D]), col(b_qog[2 * D:])
    cb, bout = col(conv_b), col(b_out)

    in_maps = []
    for c in range(N_CORES):
        b, h = c // 2, c % 2
        t0 = h * T
        xs = np.zeros((TH, DM), f)
        xs[1:T + 1] = x[b, t0:t0 + T]
        if t0 > 0:
            xs[0] = x[b, t0 - 1]
        if t0 + T < S:
            xs[T + 1] = x[b, t0 + T]
        xs = np.ascontiguousarray(xs.T.astype(bf))   # [DM, TH] feature-major
        in_maps.append({
            "x": xs, "wqt": wqt, "wct": wct, "wat": wat, "wot": wot,
            "bq": bq, "bo": bo, "bg": bg, "cb": cb, "bout": bout,
            "hf0": np.full((128, 1), 1.0 - h, f),
            "hf1": np.full((128, 1), float(h), f),
        })
    return in_maps


def kernel(x, W_qog, b_qog, conv_w, conv_b, w_a, W_out, b_out):
    nc = _build(5)
    in_maps = make_in_maps(x, W_qog, b_qog, conv_w, conv_b, w_a, W_out, b_out)
    res = None
    for attempt in range(3):
        try:
            res = run_bass_kernel_spmd(nc, in_maps, list(range(N_CORES)))
            break
        except Exception:
            # the execution path through the device bridge is occasionally
            # flaky (worker hangup); reset the backend and retry
            if attempt == 2:
                raise
            import jax

            try:
                jax.clear_backends()
            except Exception:
                pass
            import time

            time.sleep(5)
    out = np.empty((B, S, DM), np.float32)
    for c in range(N_CORES):
        b, h = c // 2, c % 2
        out[b, h * T:(h + 1) * T, :] = res.results[c]["r"].astype(np.float32).T
    return out


# revision 4
# speedup vs baseline: 1.7984x; 1.7984x over previous
"""F2NetHead Trainium2 kernel (8 NeuronCores, Bass/Tile).

Reference computation (per batch b):
    qog = x @ W_qog.T + b_qog ; Q,O,G = split(qog)
    cq  = silu(conv1d(Q, conv_w, pad=1) + conv_b)          # mixes channels
    l   = (cq @ w_a.T) / sqrt(d)
    attn= softmax(l, axis=seq)
    glob= sum_seq(Q * attn)                                 # [1, d]
    P   = O * glob
    L   = silu(G) * cumsum(P, axis=seq)
    R   = L @ W_out.T + b_out

Sharding: 8 cores = 4 batches x 2 sequence halves. Each core computes
2048 tokens of one batch. The host supplies the x-rows with a 1-token
halo on each side (zero rows at the sequence edges) so the conv needs no
neighbor exchange. The only cross-core communication is a pairwise
AllReduce of 4 small [d] vectors per batch:
    E  = sum_seq exp(l)            (softmax denominator)
    N  = sum_seq Q * exp(l)        (softmax numerator of glob)
    v  = W_O @ (sum of this half's x rows), masked to the first half
(v rides the allreduce because the matvec is linear; the second half's
cumsum offset is glob * (v + T*b_O), i.e. the first half's P-column-sums,
computed without materializing O.)

On-chip layout is feature-major ([d partitions, tokens free]) so every
sequence-axis op (softmax sums, global sum, cumsum) is a free-dim op.
All matmul operands are bf16 (full PE rate, fp32 PSUM accumulation;
~6e-3 rel err on hw — comfortably inside the 2e-2 gate) which halves
DMA traffic and SBUF residency vs fp32r. That slack is spent on:
  * weight prefetch — every phase's weights are DMA'd during phase A/B1
    so no phase start ever stalls on an HBM load;
  * collective overlap — phase C's O- and G-matmuls (which do not
    depend on the allreduce) are emitted before anything that consumes
    the allreduce result, so the PE array chews through them while the
    collective is in flight; only the cheap scan/scale/W_out tail waits.
silu is computed as x*sigmoid(x) (ACT sigmoid + DVE multiply).

Launch-path note: every ExternalInput buffer costs ~80 us of per-launch
dispatch overhead through the PJRT bridge (measured: a trivial kernel
goes 1.35 -> 2.49 ms/launch from 2 to 16 inputs). All weights are
therefore packed host-side into ONE [d_model, 8d] bf16 tensor and all
bias/flag columns into ONE [128, 42] f32 tensor, so a launch carries
just 3 inputs + 1 output.
"""

import numpy as np

import concourse.bacc as bacc
import concourse.mybir as mybir
import concourse.tile as tile
from concourse.bass_utils import run_bass_kernel_spmd

F32 = mybir.dt.float32
BF16 = mybir.dt.bfloat16
AF = mybir.ActivationFunctionType
OP = mybir.AluOpType

B, S, D, DM = 4, 4096, 1024, 1024
N_CORES = 8
T = S // 2            # tokens per core
TH = T + 2            # with halo
DT = D // 128         # d tiles (8)
KT = DM // 128        # contraction tiles (8)
ABLK = 410            # phase A token block (5 blocks over TH=2050)
BBLK = 512            # phase B token block (4 blocks over T)
CBLK = 256            # phase C token block (8 blocks over T)
NCB = T // CBLK
SCALE = 1.0 / float(np.sqrt(D))

# column offsets of the packed weight tensor w [DM, 8D]
WQ0, WO0, WG0, WA0, WU0, WC0 = 0, D, 2 * D, 3 * D, 4 * D, 5 * D
# column offsets of the packed bias tensor bsm [128, 42]
OBQ, OBO, OBG, OCB, OBU, OHF0, OHF1 = 0, DT, 2 * DT, 3 * DT, 4 * DT, 5 * DT, 5 * DT + 1
NBS = 5 * DT + 2


def _emit(tc, nc, prm, phases=5):
    reps = 1
    if phases >= 100:
        reps, phases = phases // 100, 5
    for _ in range(reps):
        _emit_once(tc, nc, prm, phases)


def _emit_once(tc, nc, prm, phases):
    x, w, bsm, r_out = prm["x"], prm["w"], prm["bsm"], prm["r"]

    with (
        tc.tile_pool(name="cols", bufs=1) as cols,
        tc.tile_pool(name="wbig", bufs=1) as wbig,
        tc.tile_pool(name="dram", bufs=1, space="DRAM") as dram,
    ):
        # phase C / matvec weights — DMA'd later (interleaved with B1's
        # conv-weight loads) so they never delay phase A's startup
        woo = wbig.tile([128, KT, DT, 128], BF16)   # W_O^T
        wog = wbig.tile([128, KT, DT, 128], BF16)   # W_G^T
        wo2 = wbig.tile([128, KT, DT, 128], BF16)   # W_out^T
        wa = wbig.tile([128, KT, DT, 128], BF16)    # w_a^T

        # all bias / flag columns in one load ([128, c] with d = a*128 + p)
        bs = cols.tile([128, NBS], F32)
        nc.sync.dma_start(bs[:], bsm[:])
        bq_sb = bs[:, OBQ:OBQ + DT]
        bo_sb = bs[:, OBO:OBO + DT]
        bg_sb = bs[:, OBG:OBG + DT]
        cb_sb = bs[:, OCB:OCB + DT]
        bout_sb = bs[:, OBU:OBU + DT]
        hf0_sb = bs[:, OHF0:OHF0 + 1]
        hf1_sb = bs[:, OHF1:OHF1 + 1]

        # accumulators that survive across phases
        sx_cols = cols.tile([128, KT, 5], F32)      # per-A-block x sums
        e_cols = cols.tile([128, DT * 4], F32)      # per-(a,B-block) exp sums
        n_cols = cols.tile([128, DT * 4], F32)      # per-(a,B-block) Q*exp sums
        stage = cols.tile([128, 4 * DT], F32)       # allreduce staging
        red = cols.tile([128, 4 * DT], F32)         # allreduce result
        glob = cols.tile([128, DT], F32)
        offset = cols.tile([128, DT], F32)

        # ---------------- phase A: Q^T over TH halo'd tokens ----------------
        with tc.tile_pool(name="qt", bufs=1) as qt_pool:
            qt = qt_pool.tile([128, DT, TH], BF16)
            with (
                tc.tile_pool(name="wq", bufs=1) as wq_pool,
                tc.tile_pool(name="xa", bufs=2) as xa_pool,
                tc.tile_pool(name="psa", bufs=8, space="PSUM") as psa,
            ):
                wq = [wq_pool.tile([128, DT, 128], BF16, tag=f"wq{kc}",
                                   name=f"wq{kc}") for kc in range(KT)]
                for kc in range(KT):
                    nc.sync.dma_start(
                        wq[kc][:],
                        w[kc * 128:(kc + 1) * 128, WQ0:WQ0 + D]
                        .rearrange("p (a m) -> p a m", m=128),
                    )
                for blk in range(5):
                    t0 = blk * ABLK
                    xt = [xa_pool.tile([128, ABLK], BF16, tag=f"xa{kc}",
                                       name=f"xa{kc}") for kc in range(KT)]
                    for kc in range(KT):
                        nc.sync.dma_start(
                            xt[kc][:],
                            x[kc * 128:(kc + 1) * 128, t0:t0 + ABLK],
                        )
                    # x column-sums over main (non-halo) tokens for cumsum offset
                    lo = 1 - t0 if t0 < 1 else 0
                    hi = ABLK - max(0, t0 + ABLK - (TH - 1))
                    for kc in range(KT):
                        nc.vector.tensor_reduce(
                            sx_cols[:, kc, blk:blk + 1], xt[kc][:, lo:hi],
                            axis=mybir.AxisListType.X, op=OP.add,
                        )
                    for a in range(DT):
                        ps = psa.tile([128, ABLK], F32, tag="ps")
                        for kc in range(KT):
                            nc.tensor.matmul(
                                ps[:], wq[kc][:, a, :], xt[kc][:],
                                start=(kc == 0), stop=(kc == KT - 1),
                            )
                        nc.vector.tensor_scalar_add(
                            qt[:, a, t0:t0 + ABLK], ps[:], bq_sb[:, a:a + 1]
                        )

            # ------------- phase B1: cq^T = silu(conv(Q)) -------------
            with tc.tile_pool(name="cq", bufs=1) as cq_pool:
                cq = cq_pool.tile([128, DT, T], BF16)
                with (
                    tc.tile_pool(name="wc", bufs=2) as wc_pool,
                    tc.tile_pool(name="psb", bufs=8, space="PSUM") as psb,
                ):
                    for a in range(DT):
                        wc = wc_pool.tile([128, 3, KT, 128], BF16, tag="wc")
                        for k3 in range(3):
                            nc.sync.dma_start(
                                wc[:, k3, :, :],
                                w[:, WC0 + k3 * D + a * 128:
                                     WC0 + k3 * D + (a + 1) * 128]
                                .rearrange("(kc p) m -> p kc m", p=128),
                            )
                        # spread the later phases' weight loads between the
                        # conv-weight loads so neither ever stalls compute
                        if a < 4:
                            w_pre = (wa, woo, wog, wo2)[a]
                            c0 = (WA0, WO0, WG0, WU0)[a]
                            for kc in range(KT):
                                nc.sync.dma_start(
                                    w_pre[:, kc, :, :],
                                    w[kc * 128:(kc + 1) * 128, c0:c0 + D]
                                    .rearrange("p (a m) -> p a m", m=128),
                                )
                        for blk in range(T // BBLK):
                            t0 = blk * BBLK
                            ps = psb.tile([128, BBLK], F32, tag="ps")
                            first = True
                            for k3 in range(3):
                                for kc in range(KT):
                                    nc.tensor.matmul(
                                        ps[:], wc[:, k3, kc, :],
                                        qt[:, kc, t0 + k3:t0 + k3 + BBLK],
                                        start=first,
                                        stop=(k3 == 2 and kc == KT - 1),
                                    )
                                    first = False
                            sig = wc_pool.tile([128, BBLK], F32, tag="sig")
                            nc.scalar.activation(
                                sig[:], ps[:], AF.Sigmoid, bias=cb_sb[:, a:a + 1]
                            )
                            nc.vector.scalar_tensor_tensor(
                                cq[:, a, t0:t0 + BBLK], ps[:], cb_sb[:, a:a + 1],
                                sig[:], OP.add, OP.mult,
                            )

                # ------- phase B2: E/N partial sums from exp(logits) -------
                with (
                    tc.tile_pool(name="ex", bufs=2) as ex_pool,
                    tc.tile_pool(name="psl", bufs=8, space="PSUM") as psl,
                ):
                    for blk in range(T // BBLK):
                        t0 = blk * BBLK
                        for a in range(DT):
                            ps = psl.tile([128, BBLK], F32, tag="ps")
                            for kc in range(KT):
                                nc.tensor.matmul(
                                    ps[:], wa[:, kc, a, :], cq[:, kc, t0:t0 + BBLK],
                                    start=(kc == 0), stop=(kc == KT - 1),
                                )
                            expl = ex_pool.tile([128, BBLK], F32, tag="expl")
                            idx = a * 4 + blk
                            nc.scalar.activation(
                                expl[:], ps[:], AF.Exp, scale=SCALE,
                                accum_out=e_cols[:, idx:idx + 1],
                            )
                            prod = ex_pool.tile([128, BBLK], F32, tag="prod")
                            nc.vector.scalar_tensor_tensor(
                                prod[:], expl[:], 0.0,
                                qt[:, a, t0 + 1:t0 + 1 + BBLK],
                                OP.add, OP.mult,
                                accum_out=n_cols[:, idx:idx + 1],
                            )

        # ------- allreduce staging: E, N, v = W_O @ (masked x sums) -------
        nc.vector.tensor_reduce(
            stage[:, 0:DT], e_cols[:].rearrange("p (a b) -> p a b", b=4),
            axis=mybir.AxisListType.X, op=OP.add,
        )
        nc.vector.tensor_reduce(
            stage[:, DT:2 * DT], n_cols[:].rearrange("p (a b) -> p a b", b=4),
            axis=mybir.AxisListType.X, op=OP.add,
        )
        # x sums (main tokens) masked to the first half: sxm = sx * (1-h)
        sxm = cols.tile([128, KT], BF16)
        sxf = cols.tile([128, KT], F32)
        nc.vector.tensor_reduce(
            sxf[:], sx_cols[:], axis=mybir.AxisListType.X, op=OP.add,
        )
        nc.vector.tensor_scalar_mul(sxm[:], sxf[:], hf0_sb[:, 0:1])
        # v = W_O @ sxm — linear, so it rides the allreduce (the matvec
        # would otherwise gate the post-collective critical path)
        with tc.tile_pool(name="psm", bufs=2, space="PSUM") as psm:
            for a in range(DT):
                ps = psm.tile([128, 1], F32, tag="ps")
                for kc in range(KT):
                    nc.tensor.matmul(
                        ps[:], woo[:, kc, a, :], sxm[:, kc:kc + 1],
                        start=(kc == 0), stop=(kc == KT - 1),
                    )
                nc.vector.tensor_scalar_add(
                    stage[:, 3 * DT + a:3 * DT + a + 1], ps[:], 0.0
                )

        if phases == 99:
            # timing-model variant: skip the collective (TimelineSim
            # cannot model collectives); copy stage -> red locally
            nc.vector.tensor_copy(red[:], stage[:])
        else:
            cc_in = dram.tile([128, 4 * DT], F32)
            cc_out = dram.tile([128, 4 * DT], F32)
            nc.sync.dma_start(cc_in[:], stage[:])
            nc.gpsimd.collective_compute(
                "AllReduce", OP.add,
                replica_groups=[[0, 1], [2, 3], [4, 5], [6, 7]],
                ins=[cc_in.opt()], outs=[cc_out.opt()],
            )
            nc.sync.dma_start(red[:], cc_out[:])

        # ---------------- phase C: O,G -> P -> cumsum -> L -> R ----------------
        # Part 1 (emitted before anything consumes `red`): O/G matmuls and
        # silu(G), staged to SBUF in bf16. The PE array runs these while the
        # allreduce is in flight.
        with (
            tc.tile_pool(name="stg", bufs=1) as stg_pool,
            tc.tile_pool(name="xc", bufs=2) as xc_pool,
            tc.tile_pool(name="blkb", bufs=2) as blk_pool,
            tc.tile_pool(name="psc", bufs=8, space="PSUM") as psc,
        ):
            ot = stg_pool.tile([128, DT, T], BF16)
            gt = stg_pool.tile([128, DT, T], BF16)
            for blk in range(NCB):
                t0 = blk * CBLK
                xt = xc_pool.tile([128, KT, CBLK], BF16, tag="xc")
                for kc in range(KT):
                    nc.sync.dma_start(
                        xt[:, kc, :],
                        x[kc * 128:(kc + 1) * 128, t0 + 1:t0 + 1 + CBLK],
                    )
                for a in range(DT):
                    ps = psc.tile([128, CBLK], F32, tag="ps")
                    for kc in range(KT):
                        nc.tensor.matmul(
                            ps[:], woo[:, kc, a, :], xt[:, kc, :],
                            start=(kc == 0), stop=(kc == KT - 1),
                        )
                    nc.scalar.copy(ot[:, a, t0:t0 + CBLK], ps[:])
                for a in range(DT):
                    ps = psc.tile([128, CBLK], F32, tag="ps")
                    for kc in range(KT):
                        nc.tensor.matmul(
                            ps[:], wog[:, kc, a, :], xt[:, kc, :],
                            start=(kc == 0), stop=(kc == KT - 1),
                        )
                    sig = xc_pool.tile([128, CBLK], F32, tag="sig")
                    nc.scalar.activation(
                        sig[:], ps[:], AF.Sigmoid, bias=bg_sb[:, a:a + 1]
                    )
                    nc.vector.scalar_tensor_tensor(
                        gt[:, a, t0:t0 + CBLK], ps[:], bg_sb[:, a:a + 1], sig[:],
                        OP.add, OP.mult,
                    )

            # Part 2 (consumes `red`): glob, cumsum offsets, then per block
            # P -> cumsum -> L -> R. Only scan/scale work plus the W_out
            # matmuls sit behind the collective.
            recip = cols.tile([128, DT], F32)
            nc.vector.reciprocal(recip[:], red[:, 0:DT])
            nc.vector.tensor_mul(glob[:], red[:, DT:2 * DT], recip[:])
            bo_t = cols.tile([128, DT], F32)
            nc.vector.tensor_scalar_mul(bo_t[:], bo_sb[:], float(T))
            offv = cols.tile([128, DT], F32)
            nc.vector.tensor_add(offv[:], red[:, 3 * DT:4 * DT], bo_t[:])
            nc.vector.tensor_mul(offset[:], offv[:], glob[:])
            nc.vector.tensor_scalar_mul(offset[:], offset[:], hf1_sb[:, 0:1])
            boglob = cols.tile([128, DT], F32)
            nc.vector.tensor_mul(boglob[:], bo_sb[:], glob[:])

            c_prev = None
            for blk in range(NCB):
                t0 = blk * CBLK
                pt = blk_pool.tile([128, DT, CBLK], F32, tag="pt")
                ct = blk_pool.tile([128, DT, CBLK], F32, tag="ct")
                carry = xc_pool.tile([128, DT], F32, tag="carry")
                lt = blk_pool.tile([128, DT, CBLK], BF16, tag="lt")
                rt = blk_pool.tile([128, DT, CBLK], BF16, tag="rt")
                for a in range(DT):
                    # P = (O + b_o) * glob = O*glob + (b_o*glob), on ACT
                    nc.scalar.activation(
                        pt[:, a, :], ot[:, a, t0:t0 + CBLK], AF.Identity,
                        bias=boglob[:, a:a + 1], scale=glob[:, a:a + 1],
                    )
                    init = (offset[:, a:a + 1] if c_prev is None
                            else c_prev[:, a:a + 1])
                    nc.vector.tensor_tensor_scan(
                        ct[:, a, :], pt[:, a, :], pt[:, a, :], init,
                        OP.add, OP.bypass,
                    )
                # carry the last cumsum column via ACT so the next
                # block's scan does not read a scan output directly
                nc.scalar.copy(carry[:], ct[:, :, CBLK - 1:CBLK])
                for a in range(DT):
                    nc.vector.tensor_mul(
                        lt[:, a, :], gt[:, a, t0:t0 + CBLK], ct[:, a, :]
                    )
                for a in range(DT):
                    ps = psc.tile([128, CBLK], F32, tag="ps")
                    for kc in range(KT):
                        nc.tensor.matmul(
                            ps[:], wo2[:, kc, a, :], lt[:, kc, :],
                            start=(kc == 0), stop=(kc == KT - 1),
                        )
                    nc.scalar.activation(
                        rt[:, a, :], ps[:], AF.Identity,
                        bias=bout_sb[:, a:a + 1],
                    )
                for a in range(DT):
                    nc.sync.dma_start(
                        r_out[a * 128:(a + 1) * 128, t0:t0 + CBLK],
                        rt[:, a, :],
                    )
                c_prev = carry


_CACHE = {}


def _build(phases=5):
    if phases in _CACHE:
        return _CACHE[phases]
    nc = bacc.Bacc(None, target_bir_lowering=False, num_devices=N_CORES)
    prm = {
        "x": nc.declare_dram_parameter("x", [DM, TH], BF16, isOutput=False),
        "w": nc.declare_dram_parameter("w", [DM, 8 * D], BF16, isOutput=False),
        "bsm": nc.declare_dram_parameter("bsm", [128, NBS], F32, isOutput=False),
        "r": nc.declare_dram_parameter("r", [DM, T], BF16, isOutput=True),
    }
    with tile.TileContext(nc, num_cores=N_CORES) as tc:
        _emit(tc, nc, prm, phases)
    nc.compile()
    _CACHE[phases] = nc
    return nc


def make_in_maps(x, W_qog, b_qog, conv_w, conv_b, w_a, W_out, b_out):
    f = np.float32
    bf = mybir.dt.np(BF16)
    x = np.asarray(x, f)
    wqt = np.asarray(W_qog, f).T                     # [dm, 3d] (WQ|WO|WG)
    wat = np.asarray(w_a, f).T
    wot = np.asarray(W_out, f).T
    cw = np.asarray(conv_w, f)
    # packed weights: [dm, 8d] = WQ | WO | WG | w_a | W_out | conv k=0,1,2
    w_all = np.concatenate(
        [wqt, wat, wot, cw[:, :, 0].T, cw[:, :, 1].T, cw[:, :, 2].T], axis=1
    ).astype(bf)
    w_all = np.ascontiguousarray(w_all)

    def col(v):  # [d] -> [128, DT] with d = a*128 + p
        return np.asarray(v, f).reshape(DT, 128).T

    b_qog = np.asarray(b_qog, f)
    bsm0 = np.concatenate(
        [col(b_qog[:D]), col(b_qog[D:2 * D]), col(b_qog[2 * D:]),
         col(conv_b), col(b_out)], axis=1
    )

    in_maps = []
    for c in range(N_CORES):
        b, h = c // 2, c % 2
        t0 = h * T
        xs = np.zeros((TH, DM), f)
        xs[1:T + 1] = x[b, t0:t0 + T]
        if t0 > 0:
            xs[0] = x[b, t0 - 1]
        if t0 + T < S:
            xs[T + 1] = x[b, t0 + T]
        xs = np.ascontiguousarray(xs.T.astype(bf))   # [DM, TH] feature-major
        bsm = np.concatenate(
            [bsm0,
             np.full((128, 1), 1.0 - h, f),
             np.full((128, 1), float(h), f)], axis=1
        )
        in_maps.append({
            "x": xs, "w": w_all, "bsm": np.ascontiguousarray(bsm),
        })
    return in_maps


def kernel(x, W_qog, b_qog, conv_w, conv_b, w_a, W_out, b_out):
    nc = _build(5)
    in_maps = make_in_maps(x, W_qog, b_qog, conv_w, conv_b, w_a, W_out, b_out)
    res = None
    for attempt in range(3):
        try:
            res = run_bass_kernel_spmd(nc, in_maps, list(range(N_CORES)))
            break
        except Exception:
            # the execution path through the device bridge is occasionally
            # flaky (worker hangup); reset the backend and retry
            if attempt == 2:
                raise
            import jax

            try:
                jax.clear_backends()
            except Exception:
                pass
            import time

            time.sleep(5)
    out = np.empty((B, S, DM), np.float32)
    for c in range(N_CORES):
        b, h = c // 2, c % 2
        out[b, h * T:(h + 1) * T, :] = res.results[c]["r"].astype(np.float32).T
    return out


# revision 9
# speedup vs baseline: 2.4400x; 1.3568x over previous
"""F2NetHead Trainium2 kernel (8 NeuronCores, Bass/Tile).

Reference computation (per batch b):
    qog = x @ W_qog.T + b_qog ; Q,O,G = split(qog)
    cq  = silu(conv1d(Q, conv_w, pad=1) + conv_b)          # mixes channels
    l   = (cq @ w_a.T) / sqrt(d)
    attn= softmax(l, axis=seq)
    glob= sum_seq(Q * attn)                                 # [1, d]
    P   = O * glob
    L   = silu(G) * cumsum(P, axis=seq)
    R   = L @ W_out.T + b_out

Sharding: 8 cores = 4 batches x 2 sequence halves. Each core computes
2048 tokens of one batch. The host supplies the x-rows with a 1-token
halo on each side (zero rows at the sequence edges) so the conv needs no
neighbor exchange. The only cross-core communication is a pairwise
AllReduce of 4 small [d] vectors per batch:
    E  = sum_seq exp(l)            (softmax denominator)
    N  = sum_seq Q * exp(l)        (softmax numerator of glob)
    v  = W_O @ (sum of this half's x rows), masked to the first half
(v rides the allreduce because the matvec is linear; the second half's
cumsum offset is glob * (v + T*b_O), i.e. the first half's P-column-sums,
computed without materializing O.)

On-chip layout is feature-major ([d partitions, tokens free]) so every
sequence-axis op (softmax sums, global sum, cumsum) is a free-dim op.
All matmul operands are bf16 (full PE rate, fp32 PSUM accumulation;
~6e-3 rel err on hw — comfortably inside the 2e-2 gate) which halves
DMA traffic and SBUF residency vs fp32r. That slack is spent on:
  * weight prefetch — every phase's weights are DMA'd during phase A/B1
    so no phase start ever stalls on an HBM load;
  * collective overlap — phase C's O- and G-matmuls (which do not
    depend on the allreduce) are emitted before anything that consumes
    the allreduce result, so the PE array chews through them while the
    collective is in flight; only the cheap scan/scale/W_out tail waits.
silu is computed as x*sigmoid(x) (ACT sigmoid + DVE multiply).

Launch-path note: every ExternalInput buffer costs ~80 us of per-launch
dispatch overhead through the PJRT bridge (measured: a trivial kernel
goes 1.35 -> 2.49 ms/launch from 2 to 16 inputs). All weights AND bias
vectors are therefore packed host-side into ONE [d_model, 8d+5] bf16
tensor, and the per-core sequence-half flags ride in 2 extra columns of
x, so a launch carries just 2 inputs + 1 output.
"""

import numpy as np

import concourse.bacc as bacc
import concourse.mybir as mybir
import concourse.tile as tile
from concourse.bass_utils import run_bass_kernel_spmd

F32 = mybir.dt.float32
BF16 = mybir.dt.bfloat16
AF = mybir.ActivationFunctionType
OP = mybir.AluOpType

B, S, D, DM = 4, 4096, 1024, 1024
N_CORES = 8
T = S // 2            # tokens per core
TH = T + 2            # with halo
DT = D // 128         # d tiles (8)
KT = DM // 128        # contraction tiles (8)
ABLK = 410            # phase A token block (5 blocks over TH=2050)
BBLK = 512            # phase B token block (4 blocks over T)
CBLK = 256            # phase C token block (8 blocks over T)
NCB = T // CBLK
SCALE = 1.0 / float(np.sqrt(D))

# column offsets of the packed weight tensor w [DM, 8D+5]; the last 5
# columns carry the bias vectors (bq|bo|bg|cb|bout, indexed by output d)
WQ0, WO0, WG0, WA0, WU0, WC0, BC0 = 0, D, 2 * D, 3 * D, 4 * D, 5 * D, 8 * D
NW = 8 * D + 5
# column offsets of the bias columns once unpacked to [128, 5*DT]
OBQ, OBO, OBG, OCB, OBU = 0, DT, 2 * DT, 3 * DT, 4 * DT
# x carries the per-core half flags in 2 extra trailing columns
NX = TH + 2


def _emit(tc, nc, prm, phases=5):
    reps = 1
    if phases >= 100:
        reps, phases = phases // 100, 5
    for _ in range(reps):
        _emit_once(tc, nc, prm, phases)


def _emit_once(tc, nc, prm, phases):
    x, w, r_out = prm["x"], prm["w"], prm["r"]

    with (
        tc.tile_pool(name="cols", bufs=1) as cols,
        tc.tile_pool(name="wbig", bufs=1) as wbig,
        tc.tile_pool(name="dram", bufs=1, space="DRAM") as dram,
    ):
        # phase C / matvec weights — DMA'd later (interleaved with B1's
        # conv-weight loads) so they never delay phase A's startup
        woo = wbig.tile([128, KT, DT, 128], BF16)   # W_O^T
        wog = wbig.tile([128, KT, DT, 128], BF16)   # W_G^T
        wo2 = wbig.tile([128, KT, DT, 128], BF16)   # W_out^T
        wa = wbig.tile([128, KT, DT, 128], BF16)    # w_a^T

        # bias / flag columns, unpacked from w / x ([128, DT], d = a*128+p)
        bsb = cols.tile([128, 5, DT], BF16)
        for j in range(5):
            nc.sync.dma_start(
                bsb[:, j, :],
                w[:, BC0 + j:BC0 + j + 1].rearrange("(a p) m -> p (a m)", p=128),
            )
        hfb = cols.tile([128, 2], BF16)
        nc.sync.dma_start(hfb[:], x[0:128, TH:TH + 2])
        bs = cols.tile([128, 5 * DT], F32)
        nc.vector.tensor_copy(bs[:], bsb[:].rearrange("p j a -> p (j a)"))
        hf = cols.tile([128, 2], F32)
        nc.vector.tensor_copy(hf[:], hfb[:])
        bq_sb = bs[:, OBQ:OBQ + DT]
        bo_sb = bs[:, OBO:OBO + DT]
        bg_sb = bs[:, OBG:OBG + DT]
        cb_sb = bs[:, OCB:OCB + DT]
        bout_sb = bs[:, OBU:OBU + DT]
        hf0_sb = hf[:, 0:1]
        hf1_sb = hf[:, 1:2]

        # accumulators that survive across phases
        sx_cols = cols.tile([128, KT, 5], F32)      # per-A-block x sums
        e_cols = cols.tile([128, DT * 4], F32)      # per-(a,B-block) exp sums
        n_cols = cols.tile([128, DT * 4], F32)      # per-(a,B-block) Q*exp sums
        stage = cols.tile([128, 4 * DT], F32)       # allreduce staging
        red = cols.tile([128, 4 * DT], F32)         # allreduce result
        glob = cols.tile([128, DT], F32)
        offset = cols.tile([128, DT], F32)

        # ---------------- phase A: Q^T over TH halo'd tokens ----------------
        with tc.tile_pool(name="qt", bufs=1) as qt_pool:
            qt = qt_pool.tile([128, DT, TH], BF16)
            with (
                tc.tile_pool(name="wq", bufs=1) as wq_pool,
                tc.tile_pool(name="xa", bufs=2) as xa_pool,
                tc.tile_pool(name="psa", bufs=8, space="PSUM") as psa,
            ):
                wq = [wq_pool.tile([128, DT, 128], BF16, tag=f"wq{kc}",
                                   name=f"wq{kc}") for kc in range(KT)]
                for kc in range(KT):
                    nc.sync.dma_start(
                        wq[kc][:],
                        w[kc * 128:(kc + 1) * 128, WQ0:WQ0 + D]
                        .rearrange("p (a m) -> p a m", m=128),
                    )
                for blk in range(5):
                    t0 = blk * ABLK
                    xt = [xa_pool.tile([128, ABLK], BF16, tag=f"xa{kc}",
                                       name=f"xa{kc}") for kc in range(KT)]
                    for kc in range(KT):
                        nc.sync.dma_start(
                            xt[kc][:],
                            x[kc * 128:(kc + 1) * 128, t0:t0 + ABLK],
                        )
                    # x column-sums over main (non-halo) tokens for cumsum offset
                    lo = 1 - t0 if t0 < 1 else 0
                    hi = ABLK - max(0, t0 + ABLK - (TH - 1))
                    for kc in range(KT):
                        nc.vector.tensor_reduce(
                            sx_cols[:, kc, blk:blk + 1], xt[kc][:, lo:hi],
                            axis=mybir.AxisListType.X, op=OP.add,
                        )
                    for a in range(DT):
                        ps = psa.tile([128, ABLK], F32, tag="ps")
                        for kc in range(KT):
                            nc.tensor.matmul(
                                ps[:], wq[kc][:, a, :], xt[kc][:],
                                start=(kc == 0), stop=(kc == KT - 1),
                            )
                        nc.vector.tensor_scalar_add(
                            qt[:, a, t0:t0 + ABLK], ps[:], bq_sb[:, a:a + 1]
                        )

            # ------------- phase B1: cq^T = silu(conv(Q)) -------------
            with tc.tile_pool(name="cq", bufs=1) as cq_pool:
                cq = cq_pool.tile([128, DT, T], BF16)
                with (
                    tc.tile_pool(name="wc", bufs=2) as wc_pool,
                    tc.tile_pool(name="psb", bufs=8, space="PSUM") as psb,
                ):
                    for a in range(DT):
                        wc = wc_pool.tile([128, 3, KT, 128], BF16, tag="wc")
                        for k3 in range(3):
                            nc.sync.dma_start(
                                wc[:, k3, :, :],
                                w[:, WC0 + k3 * D + a * 128:
                                     WC0 + k3 * D + (a + 1) * 128]
                                .rearrange("(kc p) m -> p kc m", p=128),
                            )
                        # spread the later phases' weight loads between the
                        # conv-weight loads so neither ever stalls compute
                        if a < 4:
                            w_pre = (wa, woo, wog, wo2)[a]
                            c0 = (WA0, WO0, WG0, WU0)[a]
                            for kc in range(KT):
                                nc.sync.dma_start(
                                    w_pre[:, kc, :, :],
                                    w[kc * 128:(kc + 1) * 128, c0:c0 + D]
                                    .rearrange("p (a m) -> p a m", m=128),
                                )
                        for blk in range(T // BBLK):
                            t0 = blk * BBLK
                            ps = psb.tile([128, BBLK], F32, tag="ps")
                            first = True
                            for k3 in range(3):
                                for kc in range(KT):
                                    nc.tensor.matmul(
                                        ps[:], wc[:, k3, kc, :],
                                        qt[:, kc, t0 + k3:t0 + k3 + BBLK],
                                        start=first,
                                        stop=(k3 == 2 and kc == KT - 1),
                                    )
                                    first = False
                            sig = wc_pool.tile([128, BBLK], F32, tag="sig")
                            nc.scalar.activation(
                                sig[:], ps[:], AF.Sigmoid, bias=cb_sb[:, a:a + 1]
                            )
                            nc.vector.scalar_tensor_tensor(
                                cq[:, a, t0:t0 + BBLK], ps[:], cb_sb[:, a:a + 1],
                                sig[:], OP.add, OP.mult,
                            )

                # ------- phase B2: E/N partial sums from exp(logits) -------
                with (
                    tc.tile_pool(name="ex", bufs=2) as ex_pool,
                    tc.tile_pool(name="psl", bufs=8, space="PSUM") as psl,
                ):
                    for blk in range(T // BBLK):
                        t0 = blk * BBLK
                        for a in range(DT):
                            ps = psl.tile([128, BBLK], F32, tag="ps")
                            for kc in range(KT):
                                nc.tensor.matmul(
                                    ps[:], wa[:, kc, a, :], cq[:, kc, t0:t0 + BBLK],
                                    start=(kc == 0), stop=(kc == KT - 1),
                                )
                            expl = ex_pool.tile([128, BBLK], F32, tag="expl")
                            idx = a * 4 + blk
                            nc.scalar.activation(
                                expl[:], ps[:], AF.Exp, scale=SCALE,
                                accum_out=e_cols[:, idx:idx + 1],
                            )
                            prod = ex_pool.tile([128, BBLK], F32, tag="prod")
                            nc.vector.scalar_tensor_tensor(
                                prod[:], expl[:], 0.0,
                                qt[:, a, t0 + 1:t0 + 1 + BBLK],
                                OP.add, OP.mult,
                                accum_out=n_cols[:, idx:idx + 1],
                            )

        # ------- allreduce staging: E, N, v = W_O @ (masked x sums) -------
        nc.vector.tensor_reduce(
            stage[:, 0:DT], e_cols[:].rearrange("p (a b) -> p a b", b=4),
            axis=mybir.AxisListType.X, op=OP.add,
        )
        nc.vector.tensor_reduce(
            stage[:, DT:2 * DT], n_cols[:].rearrange("p (a b) -> p a b", b=4),
            axis=mybir.AxisListType.X, op=OP.add,
        )
        # x sums (main tokens) masked to the first half: sxm = sx * (1-h)
        sxm = cols.tile([128, KT], BF16)
        sxf = cols.tile([128, KT], F32)
        nc.vector.tensor_reduce(
            sxf[:], sx_cols[:], axis=mybir.AxisListType.X, op=OP.add,
        )
        nc.vector.tensor_scalar_mul(sxm[:], sxf[:], hf0_sb[:, 0:1])
        # v = W_O @ sxm — linear, so it rides the allreduce (the matvec
        # would otherwise gate the post-collective critical path)
        with tc.tile_pool(name="psm", bufs=2, space="PSUM") as psm:
            for a in range(DT):
                ps = psm.tile([128, 1], F32, tag="ps")
                for kc in range(KT):
                    nc.tensor.matmul(
                        ps[:], woo[:, kc, a, :], sxm[:, kc:kc + 1],
                        start=(kc == 0), stop=(kc == KT - 1),
                    )
                nc.vector.tensor_scalar_add(
                    stage[:, 3 * DT + a:3 * DT + a + 1], ps[:], 0.0
                )

        if phases == 99:
            # timing-model variant: skip the collective (TimelineSim
            # cannot model collectives); copy stage -> red locally
            nc.vector.tensor_copy(red[:], stage[:])
        else:
            cc_in = dram.tile([128, 4 * DT], F32)
            cc_out = dram.tile([128, 4 * DT], F32)
            nc.sync.dma_start(cc_in[:], stage[:])
            nc.gpsimd.collective_compute(
                "AllReduce", OP.add,
                replica_groups=[[0, 1], [2, 3], [4, 5], [6, 7]],
                ins=[cc_in.opt()], outs=[cc_out.opt()],
            )
            nc.sync.dma_start(red[:], cc_out[:])

        # ---------------- phase C: O,G -> P -> cumsum -> L -> R ----------------
        # Part 1 (emitted before anything consumes `red`): O/G matmuls and
        # silu(G), staged to SBUF in bf16. The PE array runs these while the
        # allreduce is in flight.
        with (
            tc.tile_pool(name="stg", bufs=1) as stg_pool,
            tc.tile_pool(name="xc", bufs=2) as xc_pool,
            tc.tile_pool(name="blkb", bufs=2) as blk_pool,
            tc.tile_pool(name="psc", bufs=8, space="PSUM") as psc,
        ):
            ot = stg_pool.tile([128, DT, T], BF16)
            gt = stg_pool.tile([128, DT, T], BF16)
            for blk in range(NCB):
                t0 = blk * CBLK
                xt = xc_pool.tile([128, KT, CBLK], BF16, tag="xc")
                for kc in range(KT):
                    nc.sync.dma_start(
                        xt[:, kc, :],
                        x[kc * 128:(kc + 1) * 128, t0 + 1:t0 + 1 + CBLK],
                    )
                for a in range(DT):
                    ps = psc.tile([128, CBLK], F32, tag="ps")
                    for kc in range(KT):
                        nc.tensor.matmul(
                            ps[:], woo[:, kc, a, :], xt[:, kc, :],
                            start=(kc == 0), stop=(kc == KT - 1),
                        )
                    nc.scalar.copy(ot[:, a, t0:t0 + CBLK], ps[:])
                for a in range(DT):
                    ps = psc.tile([128, CBLK], F32, tag="ps")
                    for kc in range(KT):
                        nc.tensor.matmul(
                            ps[:], wog[:, kc, a, :], xt[:, kc, :],
                            start=(kc == 0), stop=(kc == KT - 1),
                        )
                    sig = xc_pool.tile([128, CBLK], F32, tag="sig")
                    nc.scalar.activation(
                        sig[:], ps[:], AF.Sigmoid, bias=bg_sb[:, a:a + 1]
                    )
                    nc.vector.scalar_tensor_tensor(
                        gt[:, a, t0:t0 + CBLK], ps[:], bg_sb[:, a:a + 1], sig[:],
                        OP.add, OP.mult,
                    )

            # Part 2 (consumes `red`): glob, cumsum offsets, then per block
            # P -> cumsum -> L -> R. Only scan/scale work plus the W_out
            # matmuls sit behind the collective.
            recip = cols.tile([128, DT], F32)
            nc.vector.reciprocal(recip[:], red[:, 0:DT])
            nc.vector.tensor_mul(glob[:], red[:, DT:2 * DT], recip[:])
            bo_t = cols.tile([128, DT], F32)
            nc.vector.tensor_scalar_mul(bo_t[:], bo_sb[:], float(T))
            offv = cols.tile([128, DT], F32)
            nc.vector.tensor_add(offv[:], red[:, 3 * DT:4 * DT], bo_t[:])
            nc.vector.tensor_mul(offset[:], offv[:], glob[:])
            nc.vector.tensor_scalar_mul(offset[:], offset[:], hf1_sb[:, 0:1])
            boglob = cols.tile([128, DT], F32)
            nc.vector.tensor_mul(boglob[:], bo_sb[:], glob[:])

            c_prev = None
            for blk in range(NCB):
                t0 = blk * CBLK
                pt = blk_pool.tile([128, DT, CBLK], F32, tag="pt")
                ct = blk_pool.tile([128, DT, CBLK], F32, tag="ct")
                carry = xc_pool.tile([128, DT], F32, tag="carry")
                lt = blk_pool.tile([128, DT, CBLK], BF16, tag="lt")
                rt = blk_pool.tile([128, DT, CBLK], BF16, tag="rt")
                for a in range(DT):
                    # P = (O + b_o) * glob = O*glob + (b_o*glob), on ACT
                    nc.scalar.activation(
                        pt[:, a, :], ot[:, a, t0:t0 + CBLK], AF.Identity,
                        bias=boglob[:, a:a + 1], scale=glob[:, a:a + 1],
                    )
                    init = (offset[:, a:a + 1] if c_prev is None
                            else c_prev[:, a:a + 1])
                    nc.vector.tensor_tensor_scan(
                        ct[:, a, :], pt[:, a, :], pt[:, a, :], init,
                        OP.add, OP.bypass,
                    )
                # carry the last cumsum column via ACT so the next
                # block's scan does not read a scan output directly
                nc.scalar.copy(carry[:], ct[:, :, CBLK - 1:CBLK])
                for a in range(DT):
                    nc.vector.tensor_mul(
                        lt[:, a, :], gt[:, a, t0:t0 + CBLK], ct[:, a, :]
                    )
                for a in range(DT):
                    ps = psc.tile([128, CBLK], F32, tag="ps")
                    for kc in range(KT):
                        nc.tensor.matmul(
                            ps[:], wo2[:, kc, a, :], lt[:, kc, :],
                            start=(kc == 0), stop=(kc == KT - 1),
                        )
                    nc.scalar.activation(
                        rt[:, a, :], ps[:], AF.Identity,
                        bias=bout_sb[:, a:a + 1],
                    )
                for a in range(DT):
                    nc.sync.dma_start(
                        r_out[a * 128:(a + 1) * 128, t0:t0 + CBLK],
                        rt[:, a, :],
                    )
                c_prev = carry


_CACHE = {}


def _build(phases=5):
    if phases in _CACHE:
        return _CACHE[phases]
    nc = bacc.Bacc(None, target_bir_lowering=False, num_devices=N_CORES)
    prm = {
        "x": nc.declare_dram_parameter("x", [DM, NX], BF16, isOutput=False),
        "w": nc.declare_dram_parameter("w", [DM, NW], BF16, isOutput=False),
        "r": nc.declare_dram_parameter("r", [DM, T], BF16, isOutput=True),
    }
    with tile.TileContext(nc, num_cores=N_CORES) as tc:
        _emit(tc, nc, prm, phases)
    nc.compile()
    _CACHE[phases] = nc
    return nc


def make_in_maps(x, W_qog, b_qog, conv_w, conv_b, w_a, W_out, b_out):
    f = np.float32
    bf = mybir.dt.np(BF16)
    x = np.asarray(x, f)
    wqt = np.asarray(W_qog, f).T                     # [dm, 3d] (WQ|WO|WG)
    wat = np.asarray(w_a, f).T
    wot = np.asarray(W_out, f).T
    cw = np.asarray(conv_w, f)
    b_qog = np.asarray(b_qog, f)
    # packed weights: [dm, 8d+5] =
    #   WQ | WO | WG | w_a | W_out | conv k=0,1,2 | bias columns
    bias_cols = np.stack(
        [b_qog[:D], b_qog[D:2 * D], b_qog[2 * D:],
         np.asarray(conv_b, f), np.asarray(b_out, f)], axis=1
    )
    w_all = np.concatenate(
        [wqt, wat, wot, cw[:, :, 0].T, cw[:, :, 1].T, cw[:, :, 2].T, bias_cols],
        axis=1,
    ).astype(bf)
    w_all = np.ascontiguousarray(w_all)

    in_maps = []
    for c in range(N_CORES):
        b, h = c // 2, c % 2
        t0 = h * T
        xs = np.zeros((NX, DM), f)
        xs[1:T + 1] = x[b, t0:t0 + T]
        if t0 > 0:
            xs[0] = x[b, t0 - 1]
        if t0 + T < S:
            xs[T + 1] = x[b, t0 + T]
        xs[TH] = 1.0 - h                             # hf0 flag column
        xs[TH + 1] = float(h)                        # hf1 flag column
        xs = np.ascontiguousarray(xs.T.astype(bf))   # [DM, NX] feature-major
        in_maps.append({"x": xs, "w": w_all})
    return in_maps


def kernel(x, W_qog, b_qog, conv_w, conv_b, w_a, W_out, b_out):
    nc = _build(5)
    in_maps = make_in_maps(x, W_qog, b_qog, conv_w, conv_b, w_a, W_out, b_out)
    res = None
    for attempt in range(3):
        try:
            res = run_bass_kernel_spmd(nc, in_maps, list(range(N_CORES)))
            break
        except Exception:
            # the execution path through the device bridge is occasionally
            # flaky (worker hangup); reset the backend and retry
            if attempt == 2:
                raise
            import jax

            try:
                jax.clear_backends()
            except Exception:
                pass
            import time

            time.sleep(5)
    out = np.empty((B, S, DM), np.float32)
    for c in range(N_CORES):
        b, h = c // 2, c % 2
        out[b, h * T:(h + 1) * T, :] = res.results[c]["r"].astype(np.float32).T
    return out


# revision 13
# speedup vs baseline: 2.8728x; 1.1774x over previous
"""F2NetHead Trainium2 kernel (8 NeuronCores, Bass/Tile).

Reference computation (per batch b):
    qog = x @ W_qog.T + b_qog ; Q,O,G = split(qog)
    cq  = silu(conv1d(Q, conv_w, pad=1) + conv_b)          # mixes channels
    l   = (cq @ w_a.T) / sqrt(d)
    attn= softmax(l, axis=seq)
    glob= sum_seq(Q * attn)                                 # [1, d]
    P   = O * glob
    L   = silu(G) * cumsum(P, axis=seq)
    R   = L @ W_out.T + b_out

Sharding: 8 cores = 4 batches x 2 sequence halves. Each core computes
2048 tokens of one batch. The host supplies the x-rows with a 1-token
halo on each side (zero rows at the sequence edges) so the conv needs no
neighbor exchange. The only cross-core communication is a pairwise
AllReduce of 4 small [d] vectors per batch:
    E  = sum_seq exp(l)            (softmax denominator)
    N  = sum_seq Q * exp(l)        (softmax numerator of glob)
    v  = W_O @ (sum of this half's x rows), masked to the first half
(v rides the allreduce because the matvec is linear; the second half's
cumsum offset is glob * (v + T*b_O), i.e. the first half's P-column-sums,
computed without materializing O.)

On-chip layout is feature-major ([d partitions, tokens free]) so every
sequence-axis op (softmax sums, global sum, cumsum) is a free-dim op.
All matmul operands are bf16 (full PE rate, fp32 PSUM accumulation;
~6e-3 rel err on hw — comfortably inside the 2e-2 gate) which halves
DMA traffic and SBUF residency vs fp32r. That slack is spent on:
  * weight prefetch — every phase's weights are DMA'd during phase A/B1
    so no phase start ever stalls on an HBM load;
  * collective overlap — phase C's O- and G-matmuls (which do not
    depend on the allreduce) are emitted before anything that consumes
    the allreduce result, so the PE array chews through them while the
    collective is in flight; only the cheap scan/scale/W_out tail waits.
silu is computed as x*sigmoid(x) (ACT sigmoid + DVE multiply).

Launch-path note: every ExternalInput buffer costs ~80 us of per-launch
dispatch overhead through the PJRT bridge (measured: a trivial kernel
goes 1.35 -> 2.49 ms/launch from 2 to 16 inputs). Activations, the
per-core half flags, all weights, and all bias vectors are therefore
packed host-side into ONE [d_model, NX] bf16 tensor per core, so a
launch carries exactly 1 input + 1 output.
"""

import numpy as np

import concourse.bacc as bacc
import concourse.mybir as mybir
import concourse.tile as tile
from concourse.bass_utils import run_bass_kernel_spmd

F32 = mybir.dt.float32
BF16 = mybir.dt.bfloat16
AF = mybir.ActivationFunctionType
OP = mybir.AluOpType

B, S, D, DM = 4, 4096, 1024, 1024
N_CORES = 8
T = S // 2            # tokens per core
TH = T + 2            # with halo
DT = D // 128         # d tiles (8)
KT = DM // 128        # contraction tiles (8)
ABLK = 410            # phase A token block (5 blocks over TH=2050)
BBLK = 512            # phase B token block (4 blocks over T)
CBLK = 256            # phase C token block (8 blocks over T)
NCB = T // CBLK
SCALE = 1.0 / float(np.sqrt(D))

# x is ONE packed [DM, NX] tensor per core:
#   cols [0, TH)        halo'd activation rows
#   cols [TH, TH+2)     per-core half flags hf0|hf1
#   cols [XW0, XW0+8D)  weights WQ | WO | WG | w_a | W_out | conv k=0,1,2
#   cols [BC0, BC0+5)   bias vectors bq|bo|bg|cb|bout (indexed by output d)
XW0 = TH + 2
WQ0, WO0, WG0 = XW0, XW0 + D, XW0 + 2 * D
WA0, WU0, WC0, BC0 = XW0 + 3 * D, XW0 + 4 * D, XW0 + 5 * D, XW0 + 8 * D
NX = XW0 + 8 * D + 5
# column offsets of the bias columns once unpacked to [128, 5*DT]
OBQ, OBO, OBG, OCB, OBU = 0, DT, 2 * DT, 3 * DT, 4 * DT


def _emit(tc, nc, prm, phases=5):
    reps = 1
    if phases >= 100:
        reps, phases = phases // 100, 5
    for _ in range(reps):
        _emit_once(tc, nc, prm, phases)


def _emit_once(tc, nc, prm, phases):
    x, r_out = prm["x"], prm["r"]

    with (
        tc.tile_pool(name="cols", bufs=1) as cols,
        tc.tile_pool(name="wbig", bufs=1) as wbig,
        tc.tile_pool(name="dram", bufs=1, space="DRAM") as dram,
    ):
        # phase C / matvec weights — DMA'd later (interleaved with B1's
        # conv-weight loads) so they never delay phase A's startup
        woo = wbig.tile([128, KT, DT, 128], BF16)   # W_O^T
        wog = wbig.tile([128, KT, DT, 128], BF16)   # W_G^T
        wo2 = wbig.tile([128, KT, DT, 128], BF16)   # W_out^T
        wa = wbig.tile([128, KT, DT, 128], BF16)    # w_a^T

        # bias / flag columns, unpacked from w / x ([128, DT], d = a*128+p)
        bsb = cols.tile([128, 5, DT], BF16)
        for j in range(5):
            nc.sync.dma_start(
                bsb[:, j, :],
                x[:, BC0 + j:BC0 + j + 1].rearrange("(a p) m -> p (a m)", p=128),
            )
        hfb = cols.tile([128, 2], BF16)
        nc.sync.dma_start(hfb[:], x[0:128, TH:TH + 2])
        bs = cols.tile([128, 5 * DT], F32)
        nc.vector.tensor_copy(bs[:], bsb[:].rearrange("p j a -> p (j a)"))
        hf = cols.tile([128, 2], F32)
        nc.vector.tensor_copy(hf[:], hfb[:])
        bq_sb = bs[:, OBQ:OBQ + DT]
        bo_sb = bs[:, OBO:OBO + DT]
        bg_sb = bs[:, OBG:OBG + DT]
        cb_sb = bs[:, OCB:OCB + DT]
        bout_sb = bs[:, OBU:OBU + DT]
        hf0_sb = hf[:, 0:1]
        hf1_sb = hf[:, 1:2]

        # accumulators that survive across phases
        sx_cols = cols.tile([128, KT, 5], F32)      # per-A-block x sums
        e_cols = cols.tile([128, DT * 4], F32)      # per-(a,B-block) exp sums
        n_cols = cols.tile([128, DT * 4], F32)      # per-(a,B-block) Q*exp sums
        stage = cols.tile([128, 4 * DT], F32)       # allreduce staging
        red = cols.tile([128, 4 * DT], F32)         # allreduce result
        glob = cols.tile([128, DT], F32)
        offset = cols.tile([128, DT], F32)

        # ---------------- phase A: Q^T over TH halo'd tokens ----------------
        with tc.tile_pool(name="qt", bufs=1) as qt_pool:
            qt = qt_pool.tile([128, DT, TH], BF16)
            with (
                tc.tile_pool(name="wq", bufs=1) as wq_pool,
                tc.tile_pool(name="xa", bufs=2) as xa_pool,
                tc.tile_pool(name="psa", bufs=8, space="PSUM") as psa,
            ):
                wq = [wq_pool.tile([128, DT, 128], BF16, tag=f"wq{kc}",
                                   name=f"wq{kc}") for kc in range(KT)]
                for kc in range(KT):
                    nc.sync.dma_start(
                        wq[kc][:],
                        x[kc * 128:(kc + 1) * 128, WQ0:WQ0 + D]
                        .rearrange("p (a m) -> p a m", m=128),
                    )
                for blk in range(5):
                    t0 = blk * ABLK
                    xt = [xa_pool.tile([128, ABLK], BF16, tag=f"xa{kc}",
                                       name=f"xa{kc}") for kc in range(KT)]
                    for kc in range(KT):
                        nc.sync.dma_start(
                            xt[kc][:],
                            x[kc * 128:(kc + 1) * 128, t0:t0 + ABLK],
                        )
                    # x column-sums over main (non-halo) tokens for cumsum offset
                    lo = 1 - t0 if t0 < 1 else 0
                    hi = ABLK - max(0, t0 + ABLK - (TH - 1))
                    for kc in range(KT):
                        nc.vector.tensor_reduce(
                            sx_cols[:, kc, blk:blk + 1], xt[kc][:, lo:hi],
                            axis=mybir.AxisListType.X, op=OP.add,
                        )
                    for a in range(DT):
                        ps = psa.tile([128, ABLK], F32, tag="ps")
                        for kc in range(KT):
                            nc.tensor.matmul(
                                ps[:], wq[kc][:, a, :], xt[kc][:],
                                start=(kc == 0), stop=(kc == KT - 1),
                            )
                        nc.vector.tensor_scalar_add(
                            qt[:, a, t0:t0 + ABLK], ps[:], bq_sb[:, a:a + 1]
                        )

            # ------------- phase B1: cq^T = silu(conv(Q)) -------------
            with tc.tile_pool(name="cq", bufs=1) as cq_pool:
                cq = cq_pool.tile([128, DT, T], BF16)
                with (
                    tc.tile_pool(name="wc", bufs=2) as wc_pool,
                    tc.tile_pool(name="psb", bufs=8, space="PSUM") as psb,
                ):
                    for a in range(DT):
                        wc = wc_pool.tile([128, 3, KT, 128], BF16, tag="wc")
                        for k3 in range(3):
                            nc.sync.dma_start(
                                wc[:, k3, :, :],
                                x[:, WC0 + k3 * D + a * 128:
                                     WC0 + k3 * D + (a + 1) * 128]
                                .rearrange("(kc p) m -> p kc m", p=128),
                            )
                        # spread the later phases' weight loads between the
                        # conv-weight loads so neither ever stalls compute
                        if a < 4:
                            w_pre = (wa, woo, wog, wo2)[a]
                            c0 = (WA0, WO0, WG0, WU0)[a]
                            for kc in range(KT):
                                nc.sync.dma_start(
                                    w_pre[:, kc, :, :],
                                    x[kc * 128:(kc + 1) * 128, c0:c0 + D]
                                    .rearrange("p (a m) -> p a m", m=128),
                                )
                        for blk in range(T // BBLK):
                            t0 = blk * BBLK
                            ps = psb.tile([128, BBLK], F32, tag="ps")
                            first = True
                            for k3 in range(3):
                                for kc in range(KT):
                                    nc.tensor.matmul(
                                        ps[:], wc[:, k3, kc, :],
                                        qt[:, kc, t0 + k3:t0 + k3 + BBLK],
                                        start=first,
                                        stop=(k3 == 2 and kc == KT - 1),
                                    )
                                    first = False
                            sig = wc_pool.tile([128, BBLK], F32, tag="sig")
                            nc.scalar.activation(
                                sig[:], ps[:], AF.Sigmoid, bias=cb_sb[:, a:a + 1]
                            )
                            nc.vector.scalar_tensor_tensor(
                                cq[:, a, t0:t0 + BBLK], ps[:], cb_sb[:, a:a + 1],
                                sig[:], OP.add, OP.mult,
                            )

                # ------- phase B2: E/N partial sums from exp(logits) -------
                with (
                    tc.tile_pool(name="ex", bufs=2) as ex_pool,
                    tc.tile_pool(name="psl", bufs=8, space="PSUM") as psl,
                ):
                    for blk in range(T // BBLK):
                        t0 = blk * BBLK
                        for a in range(DT):
                            ps = psl.tile([128, BBLK], F32, tag="ps")
                            for kc in range(KT):
                                nc.tensor.matmul(
                                    ps[:], wa[:, kc, a, :], cq[:, kc, t0:t0 + BBLK],
                                    start=(kc == 0), stop=(kc == KT - 1),
                                )
                            expl = ex_pool.tile([128, BBLK], F32, tag="expl")
                            idx = a * 4 + blk
                            nc.scalar.activation(
                                expl[:], ps[:], AF.Exp, scale=SCALE,
                                accum_out=e_cols[:, idx:idx + 1],
                            )
                            prod = ex_pool.tile([128, BBLK], F32, tag="prod")
                            nc.vector.scalar_tensor_tensor(
                                prod[:], expl[:], 0.0,
                                qt[:, a, t0 + 1:t0 + 1 + BBLK],
                                OP.add, OP.mult,
                                accum_out=n_cols[:, idx:idx + 1],
                            )

        # ------- allreduce staging: E, N, v = W_O @ (masked x sums) -------
        nc.vector.tensor_reduce(
            stage[:, 0:DT], e_cols[:].rearrange("p (a b) -> p a b", b=4),
            axis=mybir.AxisListType.X, op=OP.add,
        )
        nc.vector.tensor_reduce(
            stage[:, DT:2 * DT], n_cols[:].rearrange("p (a b) -> p a b", b=4),
            axis=mybir.AxisListType.X, op=OP.add,
        )
        # x sums (main tokens) masked to the first half: sxm = sx * (1-h)
        sxm = cols.tile([128, KT], BF16)
        sxf = cols.tile([128, KT], F32)
        nc.vector.tensor_reduce(
            sxf[:], sx_cols[:], axis=mybir.AxisListType.X, op=OP.add,
        )
        nc.vector.tensor_scalar_mul(sxm[:], sxf[:], hf0_sb[:, 0:1])
        # v = W_O @ sxm — linear, so it rides the allreduce (the matvec
        # would otherwise gate the post-collective critical path)
        with tc.tile_pool(name="psm", bufs=2, space="PSUM") as psm:
            for a in range(DT):
                ps = psm.tile([128, 1], F32, tag="ps")
                for kc in range(KT):
                    nc.tensor.matmul(
                        ps[:], woo[:, kc, a, :], sxm[:, kc:kc + 1],
                        start=(kc == 0), stop=(kc == KT - 1),
                    )
                nc.vector.tensor_scalar_add(
                    stage[:, 3 * DT + a:3 * DT + a + 1], ps[:], 0.0
                )

        if phases == 99:
            # timing-model variant: skip the collective (TimelineSim
            # cannot model collectives); copy stage -> red locally
            nc.vector.tensor_copy(red[:], stage[:])
        else:
            cc_in = dram.tile([128, 4 * DT], F32)
            cc_out = dram.tile([128, 4 * DT], F32)
            nc.sync.dma_start(cc_in[:], stage[:])
            nc.gpsimd.collective_compute(
                "AllReduce", OP.add,
                replica_groups=[[0, 1], [2, 3], [4, 5], [6, 7]],
                ins=[cc_in.opt()], outs=[cc_out.opt()],
            )
            nc.sync.dma_start(red[:], cc_out[:])

        # ---------------- phase C: O,G -> P -> cumsum -> L -> R ----------------
        # Part 1 (emitted before anything consumes `red`): O/G matmuls and
        # silu(G), staged to SBUF in bf16. The PE array runs these while the
        # allreduce is in flight.
        with (
            tc.tile_pool(name="stg", bufs=1) as stg_pool,
            tc.tile_pool(name="xc", bufs=2) as xc_pool,
            tc.tile_pool(name="blkb", bufs=2) as blk_pool,
            tc.tile_pool(name="psc", bufs=8, space="PSUM") as psc,
        ):
            ot = stg_pool.tile([128, DT, T], BF16)
            gt = stg_pool.tile([128, DT, T], BF16)
            for blk in range(NCB):
                t0 = blk * CBLK
                xt = xc_pool.tile([128, KT, CBLK], BF16, tag="xc")
                for kc in range(KT):
                    nc.sync.dma_start(
                        xt[:, kc, :],
                        x[kc * 128:(kc + 1) * 128, t0 + 1:t0 + 1 + CBLK],
                    )
                for a in range(DT):
                    ps = psc.tile([128, CBLK], F32, tag="ps")
                    for kc in range(KT):
                        nc.tensor.matmul(
                            ps[:], woo[:, kc, a, :], xt[:, kc, :],
                            start=(kc == 0), stop=(kc == KT - 1),
                        )
                    nc.scalar.copy(ot[:, a, t0:t0 + CBLK], ps[:])
                for a in range(DT):
                    ps = psc.tile([128, CBLK], F32, tag="ps")
                    for kc in range(KT):
                        nc.tensor.matmul(
                            ps[:], wog[:, kc, a, :], xt[:, kc, :],
                            start=(kc == 0), stop=(kc == KT - 1),
                        )
                    sig = xc_pool.tile([128, CBLK], F32, tag="sig")
                    nc.scalar.activation(
                        sig[:], ps[:], AF.Sigmoid, bias=bg_sb[:, a:a + 1]
                    )
                    nc.vector.scalar_tensor_tensor(
                        gt[:, a, t0:t0 + CBLK], ps[:], bg_sb[:, a:a + 1], sig[:],
                        OP.add, OP.mult,
                    )

            # Part 2 (consumes `red`): glob, cumsum offsets, then per block
            # P -> cumsum -> L -> R. Only scan/scale work plus the W_out
            # matmuls sit behind the collective.
            recip = cols.tile([128, DT], F32)
            nc.vector.reciprocal(recip[:], red[:, 0:DT])
            nc.vector.tensor_mul(glob[:], red[:, DT:2 * DT], recip[:])
            bo_t = cols.tile([128, DT], F32)
            nc.vector.tensor_scalar_mul(bo_t[:], bo_sb[:], float(T))
            offv = cols.tile([128, DT], F32)
            nc.vector.tensor_add(offv[:], red[:, 3 * DT:4 * DT], bo_t[:])
            nc.vector.tensor_mul(offset[:], offv[:], glob[:])
            nc.vector.tensor_scalar_mul(offset[:], offset[:], hf1_sb[:, 0:1])
            boglob = cols.tile([128, DT], F32)
            nc.vector.tensor_mul(boglob[:], bo_sb[:], glob[:])

            c_prev = None
            for blk in range(NCB):
                t0 = blk * CBLK
                pt = blk_pool.tile([128, DT, CBLK], F32, tag="pt")
                ct = blk_pool.tile([128, DT, CBLK], F32, tag="ct")
                carry = xc_pool.tile([128, DT], F32, tag="carry")
                lt = blk_pool.tile([128, DT, CBLK], BF16, tag="lt")
                rt = blk_pool.tile([128, DT, CBLK], BF16, tag="rt")
                for a in range(DT):
                    # P = (O + b_o) * glob = O*glob + (b_o*glob), on ACT
                    nc.scalar.activation(
                        pt[:, a, :], ot[:, a, t0:t0 + CBLK], AF.Identity,
                        bias=boglob[:, a:a + 1], scale=glob[:, a:a + 1],
                    )
                    init = (offset[:, a:a + 1] if c_prev is None
                            else c_prev[:, a:a + 1])
                    nc.vector.tensor_tensor_scan(
                        ct[:, a, :], pt[:, a, :], pt[:, a, :], init,
                        OP.add, OP.bypass,
                    )
                # carry the last cumsum column via ACT so the next
                # block's scan does not read a scan output directly
                nc.scalar.copy(carry[:], ct[:, :, CBLK - 1:CBLK])
                for a in range(DT):
                    nc.vector.tensor_mul(
                        lt[:, a, :], gt[:, a, t0:t0 + CBLK], ct[:, a, :]
                    )
                for a in range(DT):
                    ps = psc.tile([128, CBLK], F32, tag="ps")
                    for kc in range(KT):
                        nc.tensor.matmul(
                            ps[:], wo2[:, kc, a, :], lt[:, kc, :],
                            start=(kc == 0), stop=(kc == KT - 1),
                        )
                    nc.scalar.activation(
                        rt[:, a, :], ps[:], AF.Identity,
                        bias=bout_sb[:, a:a + 1],
                    )
                for a in range(DT):
                    nc.sync.dma_start(
                        r_out[a * 128:(a + 1) * 128, t0:t0 + CBLK],
                        rt[:, a, :],
                    )
                c_prev = carry


_CACHE = {}


def _build(phases=5):
    if phases in _CACHE:
        return _CACHE[phases]
    nc = bacc.Bacc(None, target_bir_lowering=False, num_devices=N_CORES)
    prm = {
        "x": nc.declare_dram_parameter("x", [DM, NX], BF16, isOutput=False),
        "r": nc.declare_dram_parameter("r", [DM, T], BF16, isOutput=True),
    }
    with tile.TileContext(nc, num_cores=N_CORES) as tc:
        _emit(tc, nc, prm, phases)
    nc.compile()
    _CACHE[phases] = nc
    return nc


def make_in_maps(x, W_qog, b_qog, conv_w, conv_b, w_a, W_out, b_out):
    f = np.float32
    bf = mybir.dt.np(BF16)
    x = np.asarray(x, f)
    wqt = np.asarray(W_qog, f).T                     # [dm, 3d] (WQ|WO|WG)
    wat = np.asarray(w_a, f).T
    wot = np.asarray(W_out, f).T
    cw = np.asarray(conv_w, f)
    b_qog = np.asarray(b_qog, f)
    # packed weights: [dm, 8d+5] =
    #   WQ | WO | WG | w_a | W_out | conv k=0,1,2 | bias columns
    bias_cols = np.stack(
        [b_qog[:D], b_qog[D:2 * D], b_qog[2 * D:],
         np.asarray(conv_b, f), np.asarray(b_out, f)], axis=1
    )
    w_all = np.concatenate(
        [wqt, wat, wot, cw[:, :, 0].T, cw[:, :, 1].T, cw[:, :, 2].T, bias_cols],
        axis=1,
    ).astype(bf)
    w_all = np.ascontiguousarray(w_all)

    in_maps = []
    for c in range(N_CORES):
        b, h = c // 2, c % 2
        t0 = h * T
        xs = np.zeros((DM, NX), bf)                  # [DM, NX] feature-major
        act = np.zeros((TH + 2, DM), f)
        act[1:T + 1] = x[b, t0:t0 + T]
        if t0 > 0:
            act[0] = x[b, t0 - 1]
        if t0 + T < S:
            act[T + 1] = x[b, t0 + T]
        act[TH] = 1.0 - h                            # hf0 flag column
        act[TH + 1] = float(h)                       # hf1 flag column
        xs[:, 0:XW0] = act.T.astype(bf)
        xs[:, XW0:] = w_all
        in_maps.append({"x": np.ascontiguousarray(xs)})
    return in_maps


def kernel(x, W_qog, b_qog, conv_w, conv_b, w_a, W_out, b_out):
    nc = _build(5)
    in_maps = make_in_maps(x, W_qog, b_qog, conv_w, conv_b, w_a, W_out, b_out)
    res = None
    for attempt in range(3):
        try:
            res = run_bass_kernel_spmd(nc, in_maps, list(range(N_CORES)))
            break
        except Exception:
            # the execution path through the device bridge is occasionally
            # flaky (worker hangup); reset the backend and retry
            if attempt == 2:
                raise
            import jax

            try:
                jax.clear_backends()
            except Exception:
                pass
            import time

            time.sleep(5)
    out = np.empty((B, S, DM), np.float32)
    for c in range(N_CORES):
        b, h = c // 2, c % 2
        out[b, h * T:(h + 1) * T, :] = res.results[c]["r"].astype(np.float32).T
    return out
